# revision 30
# baseline (speedup 1.0000x reference)
"""Trainium2 Bass kernel for multi-head attention (B=4, S=2048, D=1024, H=16).

Sharding: 8 cores = 4-way batch x 2-way head-group (8 heads per core).

v2 structure (per core), aimed at keeping the PE streaming continuously:
  - scores in pair-aligned PSUM slots [128, 2, 512]: the two K=64 matmuls of a
    head pair are adjacent in program order and target row groups (0,0)/(64,0),
    so their 512-col streams overlap on the PE (row-tiled concurrency).
  - exp is split between ACT (true exp) and DVE (Schraudolph bit-trick:
    int16(round(s*184.665 + C)) whose bits are bf16(exp(s))) to keep ACT off
    the critical path.
  - AV + softmax denominator via [V|ones] col-packed 128x128 matmuls (as v1).
  - q/k projections for head-pair m+1 and (for m=3) the output projection are
    emitted inside the attention loop; all transient PSUM (v/qk/outproj) cycles
    through the same 2-bank "acc" ring as the AV accumulators.
  - output projection partials DMA straight from PSUM to DRAM; host sums the
    2 head-group cores + bias.
"""

import numpy as np
import ml_dtypes
from contextlib import ExitStack

BF16 = ml_dtypes.bfloat16

S = 2048          # sequence length
D = 1024          # model dim
DH = 64           # head dim
HL = 8            # local heads per core
HD = HL * DH      # 512 local output dims per core
NB = 4            # batch
SCALE = 1.0 / (DH ** 0.5)

KC = D // 128     # 8 contraction chunks for projections
MC = HD // 128    # 4 output-dim chunks (= head pairs) per core
IC = S // 512     # 4 query chunks of 512
JC = S // 128     # 16 key chunks of 128

# j-chunks whose exp runs on the DVE (Schraudolph) instead of ACT.
DVE_J = (1, 3, 5, 7, 9, 11, 13, 15)
EXP_A = 128.0 / np.log(2.0)   # bf16-bits slope
EXP_C = 16248.6               # calibrated for round-to-nearest, ~zero mean

_NC_CACHE = {}


def _build_nc():
    import concourse.bacc as bacc
    import concourse.tile as tile
    from concourse import mybir

    f32 = mybir.dt.float32
    i16 = mybir.dt.int16
    bf16 = mybir.dt.bfloat16
    Exp = mybir.ActivationFunctionType.Exp

    nc = bacc.Bacc("TRN2", target_bir_lowering=False, debug=False)

    xT_d = nc.dram_tensor("xT", [D, S], bf16, kind="ExternalInput")
    wqT_d = nc.dram_tensor("wqT", [D, HD], bf16, kind="ExternalInput")
    wkT_d = nc.dram_tensor("wkT", [D, HD], bf16, kind="ExternalInput")
    wvT_d = nc.dram_tensor("wvT", [D, HD], bf16, kind="ExternalInput")
    woT_d = nc.dram_tensor("woT", [HD, D], bf16, kind="ExternalInput")
    bq_d = nc.dram_tensor("bq", [128, MC], f32, kind="ExternalInput")
    bk_d = nc.dram_tensor("bk", [128, MC], f32, kind="ExternalInput")
    bv_d = nc.dram_tensor("bv", [1, HD], f32, kind="ExternalInput")
    out_d = nc.dram_tensor("out", [S, D], f32, kind="ExternalOutput")

    with tile.TileContext(nc) as tc, ExitStack() as ctx:
        import concourse.bass as bass

        consts = ctx.enter_context(tc.tile_pool(name="consts", bufs=1))
        persist = ctx.enter_context(tc.tile_pool(name="persist", bufs=1))
        accp = ctx.enter_context(tc.tile_pool(name="acc", bufs=2, space="PSUM"))
        stp = ctx.enter_context(tc.tile_pool(name="st", bufs=3, space="PSUM"))
        ep = ctx.enter_context(tc.tile_pool(name="ep", bufs=12))
        sgp = ctx.enter_context(tc.tile_pool(name="sgp", bufs=4))
        rp = ctx.enter_context(tc.tile_pool(name="rp", bufs=4))

        woT_sb = consts.tile([128, MC, D], bf16)
        for k in range(MC):
            nc.sync.dma_start(woT_sb[:, k, :], woT_d.ap()[k * 128:(k + 1) * 128, :])

        qT_sb = persist.tile([128, MC, S], bf16)
        kT_sb = persist.tile([128, MC, S], bf16)
        # V layout per (key-chunk, local head): a 128-col block. Even local
        # heads store [V_h(64) | ones(64)], odd heads [ones(64) | V_h(64)].
        # One AV matmul then produces the attention output rows and a
        # replicated softmax denominator in the other 64 rows.
        v_m = persist.tile([128, JC, HL, 128], bf16)
        avT_sb = persist.tile([128, MC, S], bf16)

        xT_sb = consts.tile([128, KC, S], bf16)
        wqT_sb = consts.tile([128, KC, HD], bf16)
        wkT_sb = consts.tile([128, KC, HD], bf16)
        wvT_sb = consts.tile([128, KC, HD], bf16)
        bq_sb = consts.tile([128, MC], f32)
        bk_sb = consts.tile([128, MC], f32)
        bvb_sb = consts.tile([128, HD], f32)  # bv broadcast across partitions

        def whole_weight_ap(dram_t):
            # [D, HD] -> [128p, KC, HD] single strided DMA (fewer issue slots
            # on the sync engine; each dma_start costs ~0.65us to issue)
            return bass.AP(tensor=dram_t.ap().tensor, offset=0,
                           ap=[[HD, 128], [128 * HD, KC], [1, HD]])

        # issue order follows the first kT unit's critical path: x chunk 0
        # and the k weights first; biases are not needed until the first
        # bias-add ~20us in
        nc.sync.dma_start(xT_sb[:, 0, :], xT_d.ap()[0:128, :])
        nc.sync.dma_start(wkT_sb[:], whole_weight_ap(wkT_d))
        nc.sync.dma_start(wqT_sb[:], whole_weight_ap(wqT_d))
        for k in range(1, KC):
            nc.sync.dma_start(xT_sb[:, k, :], xT_d.ap()[k * 128:(k + 1) * 128, :])
            if k == 2:
                nc.sync.dma_start(wvT_sb[:], whole_weight_ap(wvT_d))
        nc.sync.dma_start(bq_sb[:], bq_d.ap())
        nc.sync.dma_start(bk_sb[:], bk_d.ap())
        bv_ap = bv_d.ap()
        bv_bcast = bass.AP(tensor=bv_ap.tensor, offset=bv_ap.offset,
                           ap=[[0, 128]] + [bv_ap.ap[-1]])
        nc.sync.dma_start(bvb_sb[:], bv_bcast)

        # ones blocks (V overwrites its own half): memset emitted in
        # 4 chunks interleaved with the v units (see the m0 pre-phase)
        bvb_r = bvb_sb[:].rearrange("p (h e) -> p h e", h=HL)

        ogp = ctx.enter_context(tc.tile_pool(name="ogp", bufs=2))

        def v_unit(t, psv):
            # V in normal layout [S, local_hd]: lhsT = x^T chunk, rhs = wv^T
            tsl = slice(t * 128, (t + 1) * 128)
            for k in range(KC):
                nc.tensor.matmul(psv[:], xT_sb[:, k, tsl], wvT_sb[:, k, :],
                                 start=(k == 0), stop=(k == KC - 1))
            psv_r = psv[:].rearrange("p (h e) -> p h e", h=HL)
            nc.vector.tensor_add(v_m[:, t, 0::2, 0:64],
                                 psv_r[:, 0::2, :], bvb_r[:, 0::2, :])
            nc.vector.tensor_add(v_m[:, t, 1::2, 64:128],
                                 psv_r[:, 1::2, :], bvb_r[:, 1::2, :])

        def qk_unit(m, which, i, ps):
            # qT/kT in [local_hd, S]: lhsT = W^T chunk (stationary)
            w_sb, b_sb, dst = ((wqT_sb, bq_sb, qT_sb) if which == 0
                               else (wkT_sb, bk_sb, kT_sb))
            isl = slice(i * 512, (i + 1) * 512)
            msl = slice(m * 128, (m + 1) * 128)
            for k in range(KC):
                nc.tensor.matmul(ps[:], w_sb[:, k, msl], xT_sb[:, k, isl],
                                 start=(k == 0), stop=(k == KC - 1))
            # bias-add on ACT: keeps the DVE queue clear so st-ring slots
            # release promptly (unit PSUM is slot-critical)
            nc.scalar.add(dst[:, m, isl], ps[:], b_sb[:, m:m + 1])

        def out_unit(sc, nh):
            # partial output projection rows [sc*128, (sc+1)*128), D-half nh;
            # host adds the 2 head-group cores + bias. The PSUM->SBUF staging
            # copy alternates ACT/DVE to split the load.
            ssl = slice(sc * 128, (sc + 1) * 128)
            po = accp.tile([128, 512], f32, tag="acc", name="po")
            for k2 in range(MC):
                nc.tensor.matmul(po[:], avT_sb[:, k2, ssl],
                                 woT_sb[:, k2, nh * 512:(nh + 1) * 512],
                                 start=(k2 == 0), stop=(k2 == MC - 1))
            og = ogp.tile([128, 512], f32, tag="og")
            if nh == 0:
                nc.scalar.copy(og[:], po[:])
            else:
                nc.vector.tensor_copy(og[:], po[:])
            nc.sync.dma_start(out_d.ap()[ssl, nh * 512:(nh + 1) * 512],
                              og[:])

        def acc_tile():
            # transient unit PSUM (v / qk proj / out proj): shares the st
            # ring slots; allocated at emission time so ring order follows
            # program order (unit chains stay acyclic vs score slots)
            return stp.tile([128, 512], f32, tag="st", name="unit")

        # ---- startup: kT(m0) + qT(m0,i0) so attention(m0) can begin; the 16
        # v units and remaining q projections are emitted inside (m0, i0)'s
        # j-loop so the PE interleaves them with early score/exp work. ----
        for i in range(IC):
            qk_unit(0, 1, i, acc_tile())
        qk_unit(0, 0, 0, acc_tile())

        def attention(m, i, aux, av_burst=(7, 15), dve_j=DVE_J):
            """aux: list of (j, fn) to emit at position j of the loop.
            av_burst: j positions after which pending AV matmuls flush in a
            back-to-back burst (fewer score->AV weight-reload switches).
            dve_j: which consumers run on the DVE this iteration."""
            h0 = 2 * m
            isl = slice(i * 512, (i + 1) * 512)
            avh = accp.tile([128, 512], f32, tag="acc")
            avh1 = accp.tile([128, 512], f32, tag="acc")
            aux_d = {}
            for j, fn in aux:
                aux_d.setdefault(j, []).append(fn)
            e_tiles = {}
            pend = []

            def flush_av():
                while pend:
                    j2 = pend.pop(0)
                    e2 = e_tiles.pop(j2)
                    nc.tensor.matmul(avh[:], v_m[:, j2, h0, :], e2[:, 0],
                                     start=(j2 == 0), stop=(j2 == JC - 1))
                    nc.tensor.matmul(avh1[:], v_m[:, j2, h0 + 1, :], e2[:, 1],
                                     start=(j2 == 0), stop=(j2 == JC - 1))

            for j in range(JC):
                jsl = slice(j * 128, (j + 1) * 128)
                # aux units (v / qk projections) emit BEFORE this j's score
                # slot: their ring position precedes it and AV(j) sees the
                # v_m[j] write in program order
                for fn in aux_d.get(j, ()):
                    fn()
                st = stp.tile([128, 2, 512], f32, tag="st")
                nc.tensor.matmul(st[:, 0], kT_sb[0:64, m, jsl],
                                 qT_sb[0:64, m, isl], start=True, stop=True)
                nc.tensor.matmul(st[:, 1], kT_sb[64:128, m, jsl],
                                 qT_sb[64:128, m, isl], start=True, stop=True)
                e = ep.tile([128, 2, 512], bf16, tag="e")
                if j in dve_j:
                    nc.vector.tensor_scalar(
                        e[:].bitcast(i16), st[:], EXP_A, EXP_C,
                        mybir.AluOpType.mult, mybir.AluOpType.add)
                else:
                    nc.scalar.activation(e[:], st[:], Exp)
                e_tiles[j] = e
                pend.append(j)
                if not av_burst:
                    flush_av()
                elif j in av_burst:
                    flush_av()
            flush_av()
            # epilogue: stage avh/avh1 to SBUF immediately (sg_h via ACT,
            # sg_h1 via DVE) so the 2-bank AV accumulator ring frees for the
            # next iteration's AV matmuls right away; then two [64,512]
            # reciprocals on the staged denominator halves, partition-swap
            # via SBUF->SBUF DMA, and normalize into avT.
            sg_h = sgp.tile([128, 512], f32, tag="sg")
            sg_h1 = sgp.tile([128, 512], f32, tag="sg")
            nc.scalar.copy(sg_h[:], avh[:])
            nc.vector.tensor_copy(sg_h1[:], avh1[:])
            dcomb = rp.tile([128, 512], f32, tag="r")
            nc.scalar.copy(dcomb[64:128, :], sg_h[64:128, :])
            nc.scalar.copy(dcomb[0:64, :], sg_h1[0:64, :])
            rcomb = rp.tile([128, 512], f32, tag="r")
            nc.vector.reciprocal_approx_fast(out=rcomb[:], in_=dcomb[:])
            rswap = rp.tile([128, 512], f32, tag="r")
            nc.sync.dma_start(rswap[0:64, :], rcomb[64:128, :])
            nc.sync.dma_start(rswap[64:128, :], rcomb[0:64, :])
            nc.vector.tensor_mul(avT_sb[0:64, m, isl], sg_h[0:64, :],
                                 rswap[0:64, :])
            nc.vector.tensor_mul(avT_sb[64:128, m, isl],
                                 sg_h1[64:128, :], rswap[64:128, :])

        for m in range(MC):
            for i in range(IC):
                aux = []
                av_burst = (7, 15)
                dve_j = DVE_J
                if m == 0 and i == 0:
                    # v pre-phase: all 16 v units through the (otherwise idle)
                    # acc ring before the attention loop - no score-ring slot
                    # hogging, no in-order PE block (avh is granted once v14
                    # releases, before the j-loop's first AV burst). Memset of
                    # the ones-halves is interleaved per 4-chunk group so the
                    # DVE never queues the full 13.7us up front.
                    for g in range(4):
                        nc.vector.memset(v_m[:, 4 * g:4 * g + 4], 1.0)
                        for t in range(4 * g, 4 * g + 4):
                            v_unit(t, accp.tile([128, 512], f32,
                                                tag="acc", name="vps"))
                    for jj, i2 in ((1, 1), (2, 2), (3, 3)):
                        aux.append((jj, (lambda i2=i2:
                                         qk_unit(0, 0, i2, acc_tile()))))
                if m == MC - 1 and i > 0:
                    # lag-by-one output projection for rows of span i-1,
                    # emitted before this iteration's loop through the acc
                    # ring (avh-grant then waits the last po release, which
                    # lands before the j=7 AV burst needs it)
                    for sc in range(4 * (i - 1), 4 * i):
                        for nh in range(2):
                            out_unit(sc, nh)
                attention(m, i, aux, av_burst, dve_j)
                # q/k projections for pair m+1 emitted at the iteration
                # boundary: the score ring has natural slack here and the
                # unit's slot-hold overlaps the epilogue window
                if m < MC - 1:
                    for u in (2 * i, 2 * i + 1):
                        qk_unit(m + 1, u // IC, u % IC, acc_tile())
        # tail: output projection for the last i-span
        for sc in range(4 * (IC - 1), 4 * IC):
            for nh in range(2):
                out_unit(sc, nh)

    nc.compile()
    return nc


def _get_nc():
    if "nc" not in _NC_CACHE:
        _NC_CACHE["nc"] = _build_nc()
    return _NC_CACHE["nc"]


def kernel(x, Wq, bq, Wk, bk, Wv, bv, Wo, bo):
    from concourse.bass_utils import run_bass_kernel_spmd

    x = np.asarray(x, dtype=np.float32)
    Wq = np.asarray(Wq, dtype=np.float32)
    Wk = np.asarray(Wk, dtype=np.float32)
    Wv = np.asarray(Wv, dtype=np.float32)
    Wo = np.asarray(Wo, dtype=np.float32)
    bq = np.asarray(bq, dtype=np.float32)
    bk = np.asarray(bk, dtype=np.float32)
    bv = np.asarray(bv, dtype=np.float32)
    bo = np.asarray(bo, dtype=np.float32)

    nc = _get_nc()

    in_maps = []
    for c in range(8):
        b = c // 2
        g = c % 2
        sl = slice(g * HD, (g + 1) * HD)
        in_maps.append({
            "xT": np.ascontiguousarray(x[b].T).astype(BF16),
            "wqT": np.ascontiguousarray((Wq[sl] * SCALE).T).astype(BF16),
            "wkT": np.ascontiguousarray(Wk[sl].T).astype(BF16),
            "wvT": np.ascontiguousarray(Wv[sl].T).astype(BF16),
            "woT": np.ascontiguousarray(Wo[:, sl].T).astype(BF16),
            "bq": np.ascontiguousarray((bq[sl] * SCALE).reshape(MC, 128).T),
            "bk": np.ascontiguousarray(bk[sl].reshape(MC, 128).T),
            "bv": bv[sl].reshape(1, HD).astype(np.float32),
        })

    _NC_CACHE["last_in_maps"] = in_maps
    res = run_bass_kernel_spmd(nc, in_maps, core_ids=list(range(8)))
    outs = [res.results[c]["out"] for c in range(8)]
    out = np.stack([outs[2 * b] + outs[2 * b + 1] for b in range(NB)])
    out = out + bo[None, None, :]
    return out.astype(np.float32)


# revision 31
# speedup vs baseline: 1.0258x; 1.0258x over previous
"""Trainium2 Bass kernel for multi-head attention (B=4, S=2048, D=1024, H=16).

Sharding: 8 cores = 4-way batch x 2-way head-group (8 heads per core).

v2 structure (per core), aimed at keeping the PE streaming continuously:
  - scores in pair-aligned PSUM slots [128, 2, 512]: the two K=64 matmuls of a
    head pair are adjacent in program order and target row groups (0,0)/(64,0),
    so their 512-col streams overlap on the PE (row-tiled concurrency).
  - exp is split between ACT (true exp) and DVE (Schraudolph bit-trick:
    int16(round(s*184.665 + C)) whose bits are bf16(exp(s))) to keep ACT off
    the critical path.
  - AV + softmax denominator via [V|ones] col-packed 128x128 matmuls (as v1).
  - q/k projections for head-pair m+1 and (for m=3) the output projection are
    emitted inside the attention loop; all transient PSUM (v/qk/outproj) cycles
    through the same 2-bank "acc" ring as the AV accumulators.
  - output projection partials DMA straight from PSUM to DRAM; host sums the
    2 head-group cores + bias.
"""

import numpy as np
import ml_dtypes
from contextlib import ExitStack

BF16 = ml_dtypes.bfloat16

S = 2048          # sequence length
D = 1024          # model dim
DH = 64           # head dim
HL = 8            # local heads per core
HD = HL * DH      # 512 local output dims per core
NB = 4            # batch
SCALE = 1.0 / (DH ** 0.5)

KC = D // 128     # 8 contraction chunks for projections
MC = HD // 128    # 4 output-dim chunks (= head pairs) per core
IC = S // 512     # 4 query chunks of 512
JC = S // 128     # 16 key chunks of 128

# j-chunks whose exp runs on the DVE (Schraudolph) instead of ACT.
DVE_J = (1, 3, 5, 7, 9, 11, 13, 15)
EXP_A = 128.0 / np.log(2.0)   # bf16-bits slope
EXP_C = 16248.6               # calibrated for round-to-nearest, ~zero mean

_NC_CACHE = {}


def _build_nc():
    import concourse.bacc as bacc
    import concourse.tile as tile
    from concourse import mybir

    f32 = mybir.dt.float32
    i16 = mybir.dt.int16
    bf16 = mybir.dt.bfloat16
    Exp = mybir.ActivationFunctionType.Exp

    nc = bacc.Bacc("TRN2", target_bir_lowering=False, debug=False)

    xT_d = nc.dram_tensor("xT", [D, S], bf16, kind="ExternalInput")
    wqT_d = nc.dram_tensor("wqT", [D, HD], bf16, kind="ExternalInput")
    wkT_d = nc.dram_tensor("wkT", [D, HD], bf16, kind="ExternalInput")
    wvT_d = nc.dram_tensor("wvT", [D, HD], bf16, kind="ExternalInput")
    woT_d = nc.dram_tensor("woT", [HD, D], bf16, kind="ExternalInput")
    bq_d = nc.dram_tensor("bq", [128, MC], f32, kind="ExternalInput")
    bk_d = nc.dram_tensor("bk", [128, MC], f32, kind="ExternalInput")
    bv_d = nc.dram_tensor("bv", [1, HD], f32, kind="ExternalInput")
    out_d = nc.dram_tensor("out", [S, D], f32, kind="ExternalOutput")

    with tile.TileContext(nc) as tc, ExitStack() as ctx:
        import concourse.bass as bass

        consts = ctx.enter_context(tc.tile_pool(name="consts", bufs=1))
        persist = ctx.enter_context(tc.tile_pool(name="persist", bufs=1))
        accp = ctx.enter_context(tc.tile_pool(name="acc", bufs=2, space="PSUM"))
        stp = ctx.enter_context(tc.tile_pool(name="st", bufs=3, space="PSUM"))
        ep = ctx.enter_context(tc.tile_pool(name="ep", bufs=12))
        sgp = ctx.enter_context(tc.tile_pool(name="sgp", bufs=4))
        rp = ctx.enter_context(tc.tile_pool(name="rp", bufs=4))

        woT_sb = consts.tile([128, MC, D], bf16)
        for k in range(MC):
            nc.sync.dma_start(woT_sb[:, k, :], woT_d.ap()[k * 128:(k + 1) * 128, :])

        qT_sb = persist.tile([128, MC, S], bf16)
        kT_sb = persist.tile([128, MC, S], bf16)
        # V layout per (key-chunk, local head): a 128-col block. Even local
        # heads store [V_h(64) | ones(64)], odd heads [ones(64) | V_h(64)].
        # One AV matmul then produces the attention output rows and a
        # replicated softmax denominator in the other 64 rows.
        v_m = persist.tile([128, JC, HL, 128], bf16)
        avT_sb = persist.tile([128, MC, S], bf16)

        xT_sb = consts.tile([128, KC, S], bf16)
        wqT_sb = consts.tile([128, KC, HD], bf16)
        wkT_sb = consts.tile([128, KC, HD], bf16)
        wvT_sb = consts.tile([128, KC, HD], bf16)
        bq_sb = consts.tile([128, MC], f32)
        bk_sb = consts.tile([128, MC], f32)
        bvb_sb = consts.tile([128, HD], f32)  # bv broadcast across partitions

        def whole_weight_ap(dram_t):
            # [D, HD] -> [128p, KC, HD] single strided DMA (fewer issue slots
            # on the sync engine; each dma_start costs ~0.65us to issue)
            return bass.AP(tensor=dram_t.ap().tensor, offset=0,
                           ap=[[HD, 128], [128 * HD, KC], [1, HD]])

        # DMA transfers serialize on the queue: issue in consumption order
        # (x0 + wk feed the first kT unit; wq before qT00; wv before the v
        # phase; biases last)
        nc.sync.dma_start(xT_sb[:, 0, :], xT_d.ap()[0:128, :])
        nc.sync.dma_start(wkT_sb[:], whole_weight_ap(wkT_d))
        for k in range(1, KC):
            nc.sync.dma_start(xT_sb[:, k, :], xT_d.ap()[k * 128:(k + 1) * 128, :])
            if k == 2:
                nc.sync.dma_start(wqT_sb[:], whole_weight_ap(wqT_d))
            if k == 5:
                nc.sync.dma_start(wvT_sb[:], whole_weight_ap(wvT_d))
        nc.sync.dma_start(bq_sb[:], bq_d.ap())
        nc.sync.dma_start(bk_sb[:], bk_d.ap())
        bv_ap = bv_d.ap()
        bv_bcast = bass.AP(tensor=bv_ap.tensor, offset=bv_ap.offset,
                           ap=[[0, 128]] + [bv_ap.ap[-1]])
        nc.sync.dma_start(bvb_sb[:], bv_bcast)

        # ones blocks (V overwrites its own half): memset emitted in
        # 4 chunks interleaved with the v units (see the m0 pre-phase)
        bvb_r = bvb_sb[:].rearrange("p (h e) -> p h e", h=HL)

        ogp = ctx.enter_context(tc.tile_pool(name="ogp", bufs=2))

        def v_unit(t, psv):
            # V in normal layout [S, local_hd]: lhsT = x^T chunk, rhs = wv^T
            tsl = slice(t * 128, (t + 1) * 128)
            for k in range(KC):
                nc.tensor.matmul(psv[:], xT_sb[:, k, tsl], wvT_sb[:, k, :],
                                 start=(k == 0), stop=(k == KC - 1))
            psv_r = psv[:].rearrange("p (h e) -> p h e", h=HL)
            nc.vector.tensor_add(v_m[:, t, 0::2, 0:64],
                                 psv_r[:, 0::2, :], bvb_r[:, 0::2, :])
            nc.vector.tensor_add(v_m[:, t, 1::2, 64:128],
                                 psv_r[:, 1::2, :], bvb_r[:, 1::2, :])

        def qk_unit(m, which, i, ps):
            # qT/kT in [local_hd, S]: lhsT = W^T chunk (stationary)
            w_sb, b_sb, dst = ((wqT_sb, bq_sb, qT_sb) if which == 0
                               else (wkT_sb, bk_sb, kT_sb))
            isl = slice(i * 512, (i + 1) * 512)
            msl = slice(m * 128, (m + 1) * 128)
            for k in range(KC):
                nc.tensor.matmul(ps[:], w_sb[:, k, msl], xT_sb[:, k, isl],
                                 start=(k == 0), stop=(k == KC - 1))
            # bias-add on ACT: keeps the DVE queue clear so st-ring slots
            # release promptly (unit PSUM is slot-critical)
            nc.scalar.add(dst[:, m, isl], ps[:], b_sb[:, m:m + 1])

        def out_unit(sc, nh):
            # partial output projection rows [sc*128, (sc+1)*128), D-half nh;
            # host adds the 2 head-group cores + bias. The PSUM->SBUF staging
            # copy alternates ACT/DVE to split the load.
            ssl = slice(sc * 128, (sc + 1) * 128)
            po = acc_tile()
            for k2 in range(MC):
                nc.tensor.matmul(po[:], avT_sb[:, k2, ssl],
                                 woT_sb[:, k2, nh * 512:(nh + 1) * 512],
                                 start=(k2 == 0), stop=(k2 == MC - 1))
            og = ogp.tile([128, 512], f32, tag="og")
            if nh == 0:
                nc.scalar.copy(og[:], po[:])
            else:
                nc.vector.tensor_copy(og[:], po[:])
            nc.sync.dma_start(out_d.ap()[ssl, nh * 512:(nh + 1) * 512],
                              og[:])

        def acc_tile():
            # transient unit PSUM (v / qk proj / out proj): shares the st
            # ring slots; allocated at emission time so ring order follows
            # program order (unit chains stay acyclic vs score slots)
            return stp.tile([128, 512], f32, tag="st", name="unit")

        # ---- startup: kT(m0) + qT(m0,i0) so attention(m0) can begin; the 16
        # v units and remaining q projections are emitted inside (m0, i0)'s
        # j-loop so the PE interleaves them with early score/exp work. ----
        for i in range(IC):
            qk_unit(0, 1, i, acc_tile())
        qk_unit(0, 0, 0, acc_tile())

        def attention(m, i, aux, av_burst=(7, 15), dve_j=DVE_J):
            """aux: list of (j, fn) to emit at position j of the loop.
            av_burst: j positions after which pending AV matmuls flush in a
            back-to-back burst (fewer score->AV weight-reload switches).
            dve_j: which consumers run on the DVE this iteration."""
            h0 = 2 * m
            isl = slice(i * 512, (i + 1) * 512)
            avh = accp.tile([128, 512], f32, tag="acc")
            avh1 = accp.tile([128, 512], f32, tag="acc")
            aux_d = {}
            for j, fn in aux:
                aux_d.setdefault(j, []).append(fn)
            e_tiles = {}
            pend = []

            def flush_av():
                while pend:
                    j2 = pend.pop(0)
                    e2 = e_tiles.pop(j2)
                    nc.tensor.matmul(avh[:], v_m[:, j2, h0, :], e2[:, 0],
                                     start=(j2 == 0), stop=(j2 == JC - 1))
                    nc.tensor.matmul(avh1[:], v_m[:, j2, h0 + 1, :], e2[:, 1],
                                     start=(j2 == 0), stop=(j2 == JC - 1))

            for j in range(JC):
                jsl = slice(j * 128, (j + 1) * 128)
                # aux units (v / qk projections) emit BEFORE this j's score
                # slot: their ring position precedes it and AV(j) sees the
                # v_m[j] write in program order
                for fn in aux_d.get(j, ()):
                    fn()
                st = stp.tile([128, 2, 512], f32, tag="st")
                nc.tensor.matmul(st[:, 0], kT_sb[0:64, m, jsl],
                                 qT_sb[0:64, m, isl], start=True, stop=True)
                nc.tensor.matmul(st[:, 1], kT_sb[64:128, m, jsl],
                                 qT_sb[64:128, m, isl], start=True, stop=True)
                e = ep.tile([128, 2, 512], bf16, tag="e")
                if j in dve_j:
                    nc.vector.tensor_scalar(
                        e[:].bitcast(i16), st[:], EXP_A, EXP_C,
                        mybir.AluOpType.mult, mybir.AluOpType.add)
                else:
                    nc.scalar.activation(e[:], st[:], Exp)
                e_tiles[j] = e
                pend.append(j)
                if not av_burst:
                    flush_av()
                elif j in av_burst:
                    flush_av()
            flush_av()
            # epilogue: stage avh/avh1 to SBUF immediately (sg_h via ACT,
            # sg_h1 via DVE) so the 2-bank AV accumulator ring frees for the
            # next iteration's AV matmuls right away; then two [64,512]
            # reciprocals on the staged denominator halves, partition-swap
            # via SBUF->SBUF DMA, and normalize into avT.
            sg_h = sgp.tile([128, 512], f32, tag="sg")
            sg_h1 = sgp.tile([128, 512], f32, tag="sg")
            nc.scalar.copy(sg_h[:], avh[:])
            nc.vector.tensor_copy(sg_h1[:], avh1[:])
            dcomb = rp.tile([128, 512], f32, tag="r")
            nc.scalar.copy(dcomb[64:128, :], sg_h[64:128, :])
            nc.scalar.copy(dcomb[0:64, :], sg_h1[0:64, :])
            rcomb = rp.tile([128, 512], f32, tag="r")
            nc.vector.reciprocal_approx_fast(out=rcomb[:], in_=dcomb[:])
            rswap = rp.tile([128, 512], f32, tag="r")
            nc.sync.dma_start(rswap[0:64, :], rcomb[64:128, :])
            nc.sync.dma_start(rswap[64:128, :], rcomb[0:64, :])
            nc.vector.tensor_mul(avT_sb[0:64, m, isl], sg_h[0:64, :],
                                 rswap[0:64, :])
            nc.vector.tensor_mul(avT_sb[64:128, m, isl],
                                 sg_h1[64:128, :], rswap[64:128, :])

        for m in range(MC):
            for i in range(IC):
                aux = []
                av_burst = (7, 15)
                dve_j = DVE_J
                if m == 0 and i == 0:
                    # v pre-phase: all 16 v units through the (otherwise idle)
                    # acc ring before the attention loop - no score-ring slot
                    # hogging, no in-order PE block (avh is granted once v14
                    # releases, before the j-loop's first AV burst). Memset of
                    # the ones-halves is interleaved per 4-chunk group so the
                    # DVE never queues the full 13.7us up front.
                    for g in range(4):
                        nc.vector.memset(v_m[:, 4 * g:4 * g + 4], 1.0)
                        for t in range(4 * g, 4 * g + 4):
                            v_unit(t, accp.tile([128, 512], f32,
                                                tag="acc", name="vps"))
                    for jj, i2 in ((1, 1), (2, 2), (3, 3)):
                        aux.append((jj, (lambda i2=i2:
                                         qk_unit(0, 0, i2, acc_tile()))))
                if m == MC - 1 and i > 0:
                    # lag-by-one output projection: rows of i-1, spread as
                    # single po units through this iteration's loop
                    for slot, sc in enumerate(range(4 * (i - 1), 4 * i)):
                        for nh in range(2):
                            aux.append((2 * slot * 2 + nh * 2 + 1,
                                        (lambda sc=sc, nh=nh:
                                         out_unit(sc, nh))))
                attention(m, i, aux, av_burst, dve_j)
                # q/k projections for pair m+1 emitted at the iteration
                # boundary: the score ring has natural slack here and the
                # unit's slot-hold overlaps the epilogue window
                if m < MC - 1:
                    for u in (2 * i, 2 * i + 1):
                        qk_unit(m + 1, u // IC, u % IC, acc_tile())
        # tail: output projection for the last i-span
        for sc in range(4 * (IC - 1), 4 * IC):
            for nh in range(2):
                out_unit(sc, nh)

    nc.compile()
    return nc


def _get_nc():
    if "nc" not in _NC_CACHE:
        _NC_CACHE["nc"] = _build_nc()
    return _NC_CACHE["nc"]


def kernel(x, Wq, bq, Wk, bk, Wv, bv, Wo, bo):
    from concourse.bass_utils import run_bass_kernel_spmd

    x = np.asarray(x, dtype=np.float32)
    Wq = np.asarray(Wq, dtype=np.float32)
    Wk = np.asarray(Wk, dtype=np.float32)
    Wv = np.asarray(Wv, dtype=np.float32)
    Wo = np.asarray(Wo, dtype=np.float32)
    bq = np.asarray(bq, dtype=np.float32)
    bk = np.asarray(bk, dtype=np.float32)
    bv = np.asarray(bv, dtype=np.float32)
    bo = np.asarray(bo, dtype=np.float32)

    nc = _get_nc()

    in_maps = []
    for c in range(8):
        b = c // 2
        g = c % 2
        sl = slice(g * HD, (g + 1) * HD)
        in_maps.append({
            "xT": np.ascontiguousarray(x[b].T).astype(BF16),
            "wqT": np.ascontiguousarray((Wq[sl] * SCALE).T).astype(BF16),
            "wkT": np.ascontiguousarray(Wk[sl].T).astype(BF16),
            "wvT": np.ascontiguousarray(Wv[sl].T).astype(BF16),
            "woT": np.ascontiguousarray(Wo[:, sl].T).astype(BF16),
            "bq": np.ascontiguousarray((bq[sl] * SCALE).reshape(MC, 128).T),
            "bk": np.ascontiguousarray(bk[sl].reshape(MC, 128).T),
            "bv": bv[sl].reshape(1, HD).astype(np.float32),
        })

    _NC_CACHE["last_in_maps"] = in_maps
    res = run_bass_kernel_spmd(nc, in_maps, core_ids=list(range(8)))
    outs = [res.results[c]["out"] for c in range(8)]
    out = np.stack([outs[2 * b] + outs[2 * b + 1] for b in range(NB)])
    out = out + bo[None, None, :]
    return out.astype(np.float32)


# revision 32
# speedup vs baseline: 1.0356x; 1.0096x over previous
"""Trainium2 Bass kernel for multi-head attention (B=4, S=2048, D=1024, H=16).

Sharding: 8 cores = 4-way batch x 2-way head-group (8 heads per core).

v2 structure (per core), aimed at keeping the PE streaming continuously:
  - scores in pair-aligned PSUM slots [128, 2, 512]: the two K=64 matmuls of a
    head pair are adjacent in program order and target row groups (0,0)/(64,0),
    so their 512-col streams overlap on the PE (row-tiled concurrency).
  - exp is split between ACT (true exp) and DVE (Schraudolph bit-trick:
    int16(round(s*184.665 + C)) whose bits are bf16(exp(s))) to keep ACT off
    the critical path.
  - AV + softmax denominator via [V|ones] col-packed 128x128 matmuls (as v1).
  - q/k projections for head-pair m+1 and (for m=3) the output projection are
    emitted inside the attention loop; all transient PSUM (v/qk/outproj) cycles
    through the same 2-bank "acc" ring as the AV accumulators.
  - output projection partials DMA straight from PSUM to DRAM; host sums the
    2 head-group cores + bias.
"""

import numpy as np
import ml_dtypes
from contextlib import ExitStack

BF16 = ml_dtypes.bfloat16

S = 2048          # sequence length
D = 1024          # model dim
DH = 64           # head dim
HL = 8            # local heads per core
HD = HL * DH      # 512 local output dims per core
NB = 4            # batch
SCALE = 1.0 / (DH ** 0.5)

KC = D // 128     # 8 contraction chunks for projections
MC = HD // 128    # 4 output-dim chunks (= head pairs) per core
IC = S // 512     # 4 query chunks of 512
JC = S // 128     # 16 key chunks of 128

# j-chunks whose exp runs on the DVE (Schraudolph) instead of ACT.
DVE_J = (1, 3, 5, 7, 9, 11, 13, 15)
EXP_A = 128.0 / np.log(2.0)   # bf16-bits slope
EXP_C = 16248.6               # calibrated for round-to-nearest, ~zero mean

_NC_CACHE = {}


def _build_nc():
    import concourse.bacc as bacc
    import concourse.tile as tile
    from concourse import mybir

    f32 = mybir.dt.float32
    i16 = mybir.dt.int16
    bf16 = mybir.dt.bfloat16
    Exp = mybir.ActivationFunctionType.Exp

    nc = bacc.Bacc("TRN2", target_bir_lowering=False, debug=False)

    xT_d = nc.dram_tensor("xT", [D, S], bf16, kind="ExternalInput")
    wqT_d = nc.dram_tensor("wqT", [D, HD], bf16, kind="ExternalInput")
    wkT_d = nc.dram_tensor("wkT", [D, HD], bf16, kind="ExternalInput")
    wvT_d = nc.dram_tensor("wvT", [D, HD], bf16, kind="ExternalInput")
    woT_d = nc.dram_tensor("woT", [HD, D], bf16, kind="ExternalInput")
    bq_d = nc.dram_tensor("bq", [128, MC], f32, kind="ExternalInput")
    bk_d = nc.dram_tensor("bk", [128, MC], f32, kind="ExternalInput")
    bv_d = nc.dram_tensor("bv", [1, HD], f32, kind="ExternalInput")
    out_d = nc.dram_tensor("out", [S, D], f32, kind="ExternalOutput")

    with tile.TileContext(nc) as tc, ExitStack() as ctx:
        import concourse.bass as bass

        consts = ctx.enter_context(tc.tile_pool(name="consts", bufs=1))
        persist = ctx.enter_context(tc.tile_pool(name="persist", bufs=1))
        accp = ctx.enter_context(tc.tile_pool(name="acc", bufs=2, space="PSUM"))
        stp = ctx.enter_context(tc.tile_pool(name="st", bufs=3, space="PSUM"))
        ep = ctx.enter_context(tc.tile_pool(name="ep", bufs=12))
        sgp = ctx.enter_context(tc.tile_pool(name="sgp", bufs=4))
        rp = ctx.enter_context(tc.tile_pool(name="rp", bufs=4))

        woT_sb = consts.tile([128, MC, D], bf16)
        for k in range(MC):
            nc.sync.dma_start(woT_sb[:, k, :], woT_d.ap()[k * 128:(k + 1) * 128, :])

        qT_sb = persist.tile([128, MC, S], bf16)
        kT_sb = persist.tile([128, MC, S], bf16)
        # V layout per (key-chunk, local head): a 128-col block. Even local
        # heads store [V_h(64) | ones(64)], odd heads [ones(64) | V_h(64)].
        # One AV matmul then produces the attention output rows and a
        # replicated softmax denominator in the other 64 rows.
        v_m = persist.tile([128, JC, HL, 128], bf16)
        avT_sb = persist.tile([128, MC, S], bf16)

        xT_sb = consts.tile([128, KC, S], bf16)
        wqT_sb = consts.tile([128, KC, HD], bf16)
        wkT_sb = consts.tile([128, KC, HD], bf16)
        wvT_sb = consts.tile([128, KC, HD], bf16)
        bq_sb = consts.tile([128, MC], f32)
        bk_sb = consts.tile([128, MC], f32)
        bvb_sb = consts.tile([128, HD], f32)  # bv broadcast across partitions

        def whole_weight_ap(dram_t):
            # [D, HD] -> [128p, KC, HD] single strided DMA (fewer issue slots
            # on the sync engine; each dma_start costs ~0.65us to issue)
            return bass.AP(tensor=dram_t.ap().tensor, offset=0,
                           ap=[[HD, 128], [128 * HD, KC], [1, HD]])

        nc.sync.dma_start(bq_sb[:], bq_d.ap())
        nc.sync.dma_start(bk_sb[:], bk_d.ap())
        nc.sync.dma_start(wkT_sb[:], whole_weight_ap(wkT_d))
        nc.sync.dma_start(wqT_sb[:], whole_weight_ap(wqT_d))
        nc.sync.dma_start(wvT_sb[:], whole_weight_ap(wvT_d))
        # xT chunked so qk-unit k-loops pipeline with chunk arrival
        for k in range(KC):
            nc.sync.dma_start(xT_sb[:, k, :], xT_d.ap()[k * 128:(k + 1) * 128, :])
        bv_ap = bv_d.ap()
        bv_bcast = bass.AP(tensor=bv_ap.tensor, offset=bv_ap.offset,
                           ap=[[0, 128]] + [bv_ap.ap[-1]])
        nc.sync.dma_start(bvb_sb[:], bv_bcast)

        # ones blocks (V overwrites its own half): memset emitted in
        # 4 chunks interleaved with the v units (see the m0 pre-phase)
        bvb_r = bvb_sb[:].rearrange("p (h e) -> p h e", h=HL)

        ogp = ctx.enter_context(tc.tile_pool(name="ogp", bufs=2))

        def v_unit(t, psv):
            # V in normal layout [S, local_hd]: lhsT = x^T chunk, rhs = wv^T
            tsl = slice(t * 128, (t + 1) * 128)
            for k in range(KC):
                nc.tensor.matmul(psv[:], xT_sb[:, k, tsl], wvT_sb[:, k, :],
                                 start=(k == 0), stop=(k == KC - 1))
            psv_r = psv[:].rearrange("p (h e) -> p h e", h=HL)
            nc.vector.tensor_add(v_m[:, t, 0::2, 0:64],
                                 psv_r[:, 0::2, :], bvb_r[:, 0::2, :])
            nc.vector.tensor_add(v_m[:, t, 1::2, 64:128],
                                 psv_r[:, 1::2, :], bvb_r[:, 1::2, :])

        def qk_unit(m, which, i, ps):
            # qT/kT in [local_hd, S]: lhsT = W^T chunk (stationary)
            w_sb, b_sb, dst = ((wqT_sb, bq_sb, qT_sb) if which == 0
                               else (wkT_sb, bk_sb, kT_sb))
            isl = slice(i * 512, (i + 1) * 512)
            msl = slice(m * 128, (m + 1) * 128)
            for k in range(KC):
                nc.tensor.matmul(ps[:], w_sb[:, k, msl], xT_sb[:, k, isl],
                                 start=(k == 0), stop=(k == KC - 1))
            # bias-add on ACT: keeps the DVE queue clear so st-ring slots
            # release promptly (unit PSUM is slot-critical)
            nc.scalar.add(dst[:, m, isl], ps[:], b_sb[:, m:m + 1])

        def out_unit(sc, nh):
            # partial output projection rows [sc*128, (sc+1)*128), D-half nh;
            # host adds the 2 head-group cores + bias. The PSUM->SBUF staging
            # copy alternates ACT/DVE to split the load.
            ssl = slice(sc * 128, (sc + 1) * 128)
            po = acc_tile()
            for k2 in range(MC):
                nc.tensor.matmul(po[:], avT_sb[:, k2, ssl],
                                 woT_sb[:, k2, nh * 512:(nh + 1) * 512],
                                 start=(k2 == 0), stop=(k2 == MC - 1))
            og = ogp.tile([128, 512], f32, tag="og")
            if nh == 0:
                nc.scalar.copy(og[:], po[:])
            else:
                nc.vector.tensor_copy(og[:], po[:])
            nc.sync.dma_start(out_d.ap()[ssl, nh * 512:(nh + 1) * 512],
                              og[:])

        def acc_tile():
            # transient unit PSUM (v / qk proj / out proj): shares the st
            # ring slots; allocated at emission time so ring order follows
            # program order (unit chains stay acyclic vs score slots)
            return stp.tile([128, 512], f32, tag="st", name="unit")

        # ---- startup: kT(m0) + qT(m0,i0) so attention(m0) can begin; the 16
        # v units and remaining q projections are emitted inside (m0, i0)'s
        # j-loop so the PE interleaves them with early score/exp work. ----
        for i in range(IC):
            qk_unit(0, 1, i, acc_tile())
        qk_unit(0, 0, 0, acc_tile())

        def attention(m, i, aux, av_burst=(7, 15), dve_j=DVE_J):
            """aux: list of (j, fn) to emit at position j of the loop.
            av_burst: j positions after which pending AV matmuls flush in a
            back-to-back burst (fewer score->AV weight-reload switches).
            dve_j: which consumers run on the DVE this iteration."""
            h0 = 2 * m
            isl = slice(i * 512, (i + 1) * 512)
            avh = accp.tile([128, 512], f32, tag="acc")
            avh1 = accp.tile([128, 512], f32, tag="acc")
            aux_d = {}
            for j, fn in aux:
                aux_d.setdefault(j, []).append(fn)
            e_tiles = {}
            pend = []

            def flush_av():
                while pend:
                    j2 = pend.pop(0)
                    e2 = e_tiles.pop(j2)
                    nc.tensor.matmul(avh[:], v_m[:, j2, h0, :], e2[:, 0],
                                     start=(j2 == 0), stop=(j2 == JC - 1))
                    nc.tensor.matmul(avh1[:], v_m[:, j2, h0 + 1, :], e2[:, 1],
                                     start=(j2 == 0), stop=(j2 == JC - 1))

            for j in range(JC):
                jsl = slice(j * 128, (j + 1) * 128)
                # aux units (v / qk projections) emit BEFORE this j's score
                # slot: their ring position precedes it and AV(j) sees the
                # v_m[j] write in program order
                for fn in aux_d.get(j, ()):
                    fn()
                st = stp.tile([128, 2, 512], f32, tag="st")
                nc.tensor.matmul(st[:, 0], kT_sb[0:64, m, jsl],
                                 qT_sb[0:64, m, isl], start=True, stop=True)
                nc.tensor.matmul(st[:, 1], kT_sb[64:128, m, jsl],
                                 qT_sb[64:128, m, isl], start=True, stop=True)
                e = ep.tile([128, 2, 512], bf16, tag="e")
                if j in dve_j:
                    nc.vector.tensor_scalar(
                        e[:].bitcast(i16), st[:], EXP_A, EXP_C,
                        mybir.AluOpType.mult, mybir.AluOpType.add)
                else:
                    nc.scalar.activation(e[:], st[:], Exp)
                e_tiles[j] = e
                pend.append(j)
                if not av_burst:
                    flush_av()
                elif j in av_burst:
                    flush_av()
            flush_av()
            # epilogue: stage avh/avh1 to SBUF immediately (sg_h via ACT,
            # sg_h1 via DVE) so the 2-bank AV accumulator ring frees for the
            # next iteration's AV matmuls right away; then two [64,512]
            # reciprocals on the staged denominator halves, partition-swap
            # via SBUF->SBUF DMA, and normalize into avT.
            sg_h = sgp.tile([128, 512], f32, tag="sg")
            sg_h1 = sgp.tile([128, 512], f32, tag="sg")
            nc.scalar.copy(sg_h[:], avh[:])
            nc.vector.tensor_copy(sg_h1[:], avh1[:])
            dcomb = rp.tile([128, 512], f32, tag="r")
            nc.scalar.copy(dcomb[64:128, :], sg_h[64:128, :])
            nc.scalar.copy(dcomb[0:64, :], sg_h1[0:64, :])
            rcomb = rp.tile([128, 512], f32, tag="r")
            nc.vector.reciprocal_approx_fast(out=rcomb[:], in_=dcomb[:])
            rswap = rp.tile([128, 512], f32, tag="r")
            nc.sync.dma_start(rswap[0:64, :], rcomb[64:128, :])
            nc.sync.dma_start(rswap[64:128, :], rcomb[0:64, :])
            nc.vector.tensor_mul(avT_sb[0:64, m, isl], sg_h[0:64, :],
                                 rswap[0:64, :])
            nc.vector.tensor_mul(avT_sb[64:128, m, isl],
                                 sg_h1[64:128, :], rswap[64:128, :])

        for m in range(MC):
            for i in range(IC):
                aux = []
                av_burst = (7, 15)
                dve_j = DVE_J
                if m == 0 and i == 0:
                    # v pre-phase: all 16 v units through the (otherwise idle)
                    # acc ring before the attention loop - no score-ring slot
                    # hogging, no in-order PE block (avh is granted once v14
                    # releases, before the j-loop's first AV burst). Memset of
                    # the ones-halves is interleaved per 4-chunk group so the
                    # DVE never queues the full 13.7us up front.
                    for g in range(4):
                        nc.vector.memset(v_m[:, 4 * g:4 * g + 4], 1.0)
                        for t in range(4 * g, 4 * g + 4):
                            v_unit(t, accp.tile([128, 512], f32,
                                                tag="acc", name="vps"))
                    for jj, i2 in ((1, 1), (2, 2), (3, 3)):
                        aux.append((jj, (lambda i2=i2:
                                         qk_unit(0, 0, i2, acc_tile()))))
                if m == MC - 1 and i > 0:
                    # lag-by-one output projection: rows of i-1, spread as
                    # single po units through this iteration's loop
                    for slot, sc in enumerate(range(4 * (i - 1), 4 * i)):
                        for nh in range(2):
                            aux.append((2 * slot * 2 + nh * 2 + 1,
                                        (lambda sc=sc, nh=nh:
                                         out_unit(sc, nh))))
                attention(m, i, aux, av_burst, dve_j)
                # q/k projections for pair m+1 emitted at the iteration
                # boundary: the score ring has natural slack here and the
                # unit's slot-hold overlaps the epilogue window
                if m < MC - 1:
                    for u in (2 * i, 2 * i + 1):
                        qk_unit(m + 1, u // IC, u % IC, acc_tile())
        # tail: output projection for the last i-span
        for sc in range(4 * (IC - 1), 4 * IC):
            for nh in range(2):
                out_unit(sc, nh)

    nc.compile()
    return nc


def _get_nc():
    if "nc" not in _NC_CACHE:
        _NC_CACHE["nc"] = _build_nc()
    return _NC_CACHE["nc"]


def kernel(x, Wq, bq, Wk, bk, Wv, bv, Wo, bo):
    from concourse.bass_utils import run_bass_kernel_spmd

    x = np.asarray(x, dtype=np.float32)
    Wq = np.asarray(Wq, dtype=np.float32)
    Wk = np.asarray(Wk, dtype=np.float32)
    Wv = np.asarray(Wv, dtype=np.float32)
    Wo = np.asarray(Wo, dtype=np.float32)
    bq = np.asarray(bq, dtype=np.float32)
    bk = np.asarray(bk, dtype=np.float32)
    bv = np.asarray(bv, dtype=np.float32)
    bo = np.asarray(bo, dtype=np.float32)

    nc = _get_nc()

    in_maps = []
    for c in range(8):
        b = c // 2
        g = c % 2
        sl = slice(g * HD, (g + 1) * HD)
        in_maps.append({
            "xT": np.ascontiguousarray(x[b].T).astype(BF16),
            "wqT": np.ascontiguousarray((Wq[sl] * SCALE).T).astype(BF16),
            "wkT": np.ascontiguousarray(Wk[sl].T).astype(BF16),
            "wvT": np.ascontiguousarray(Wv[sl].T).astype(BF16),
            "woT": np.ascontiguousarray(Wo[:, sl].T).astype(BF16),
            "bq": np.ascontiguousarray((bq[sl] * SCALE).reshape(MC, 128).T),
            "bk": np.ascontiguousarray(bk[sl].reshape(MC, 128).T),
            "bv": bv[sl].reshape(1, HD).astype(np.float32),
        })

    _NC_CACHE["last_in_maps"] = in_maps
    res = run_bass_kernel_spmd(nc, in_maps, core_ids=list(range(8)))
    outs = [res.results[c]["out"] for c in range(8)]
    out = np.stack([outs[2 * b] + outs[2 * b + 1] for b in range(NB)])
    out = out + bo[None, None, :]
    return out.astype(np.float32)
